# revision 22
# baseline (speedup 1.0000x reference)
"""Trainium2 Bass kernel for nn_ActionReselector (topk_masking).

reference:
    q = city_embed @ Wq                 [B, NC, D]
    k = agent_embed @ Wk                [B, NA, D]
    scores = q @ k.T / sqrt(D)          [B, NC, NA]
    out = argmax(10*tanh(scores), -1)   [B, NC] int32

Key identities used here:
  - tanh and the positive scales are strictly monotonic -> argmax(scores).
  - scores = city @ (Wq @ k.T) = city @ M with M = [D, NA] tiny.
So per batch we compute M once (three small matmuls), then stream city
through PE (transpose via identity matmul, then scores matmul into grouped
PSUM banks), reduce per-block maxes with 3D reduce_max, and extract indices
with DVE max_index (slot 0 = first index of max, matching jnp.argmax).

Performance structure (per core, 8 batches, ~170us):
  - city loads via SWDGE (gpsimd dma) with 5KB descriptors: engages all 16
    SDMA engines (HWDGE only engaged 5 -> was 2.4x slower).
  - 4 transposes / 4 score matmuls share one PSUM bank each -> one ACT copy
    and one 3D reduce_max per bank instead of four.
  - a rank-1 bf16 bias matmul adds 64*t to sub-block t's scores so the four
    sub-blocks in a bank occupy disjoint value ranges; then ONE max_index
    per bank finds all four argmaxes (host subtracts 100*t).  Exact: scores
    are |x| < ~4 << 64, and slot k's value first-occurs in its own range.
  - everything fp32: float32r is reduced precision, bf16 splits shift the
    argmax for hundreds of rows.  PE (fp32 transposes + matmuls) is the
    bottleneck at ~86% duty.

Sharding: data-parallel over batch B=64 across 8 cores (8 batches/core).
"""

import sys

import numpy as np

try:
    import concourse.bacc as bacc
except ImportError:  # fresh env without the repo on sys.path
    for _p in ("/opt/trn_rl_repo", "/root/.axon_site/_ro/trn_rl_repo"):
        if _p not in sys.path:
            sys.path.insert(0, _p)
    import concourse.bacc as bacc
import concourse.mybir as mybir
import concourse.tile as tile
from concourse import masks

# Problem shapes (hardcoded per contract)
B = 64
NA = 100
NC = 5000
D = 128
N_CORES = 8
B_PER_CORE = B // N_CORES

# City tiling: 4 DMAs per batch, each 1250 cities laid out as
# [125 partitions x 10 cities x 128] so each partition line is 5KB
# contiguous in DRAM.  city index c = 1250*j + 10*p + n.
NDMA = 4
CITIES_PER_DMA = NC // NDMA       # 1250
CPP = 10                          # cities per partition per DMA
P_USED = CITIES_PER_DMA // CPP    # 125
NSUB = NDMA * CPP                 # 40 sub-blocks (125 cities each) per batch
GRP = 4                           # sub-blocks per PSUM bank group
NGRP = NSUB // GRP                # 10 groups per batch

F32 = mybir.dt.float32
U32 = mybir.dt.uint32
F32R = mybir.dt.float32r
USE_F32R = False  # float32r is reduced precision (TF32-like); corrupts transposes
USE_SWDGE = True
GROUPED_MAXIDX = True
TRANS_BF16_ID = False  # bass asserts mixed fp32/bf16 matmul gives wrong output
BIAS = 64.0
AX = mybir.AxisListType

assert P_USED * GRP * 4 <= 2048        # ctT group fits one PSUM bank
assert NA * GRP * 4 <= 2048            # score group fits one PSUM bank


def build_nc(reps=1):
    nc = bacc.Bacc(None, target_bir_lowering=False)

    city = nc.dram_tensor("city", [B_PER_CORE, NC, D], F32, kind="ExternalInput")
    agent = nc.dram_tensor("agent", [B_PER_CORE, NA, D], F32, kind="ExternalInput")
    wq = nc.dram_tensor("wq", [D, D], F32, kind="ExternalInput")
    wk = nc.dram_tensor("wk", [D, D], F32, kind="ExternalInput")
    out = nc.dram_tensor("out", [B_PER_CORE, P_USED, NSUB], U32, kind="ExternalOutput")

    with tile.TileContext(nc) as tc:
        with (
            tc.tile_pool(name="const", bufs=1) as constp,
            tc.tile_pool(name="weights", bufs=1) as wp,
            tc.tile_pool(name="cityin", bufs=8) as cityp,
            tc.tile_pool(name="cityT", bufs=4) as ctp,
            tc.tile_pool(name="psumT", bufs=2, space="PSUM") as ptp,
            tc.tile_pool(name="psumCT", bufs=3, space="PSUM") as ctpp,
            tc.tile_pool(name="psumS", bufs=3, space="PSUM") as psp,
            tc.tile_pool(name="mmat", bufs=3) as mp,
            tc.tile_pool(name="small", bufs=3) as smallp,
            tc.tile_pool(name="stage", bufs=3) as stagep,
        ):
            ident = constp.tile([128, 128], F32)
            masks.make_identity(nc, ident[:])
            ident_bf = constp.tile([128, 128], mybir.dt.bfloat16)
            masks.make_identity(nc, ident_bf[:])

            BF16 = mybir.dt.bfloat16
            ones_row = constp.tile([1, P_USED], BF16)
            nc.gpsimd.memset(ones_row[:], 1.0)
            bias_row = constp.tile([1, GRP * NA], BF16)
            for _t in range(GRP):
                nc.gpsimd.memset(bias_row[:, _t * NA:(_t + 1) * NA], BIAS * _t)

            wq_sb = wp.tile([128, 128], F32)
            nc.sync.dma_start(wq_sb[:], wq[:])
            wk_sb = wp.tile([128, 128], F32)
            nc.sync.dma_start(wk_sb[:], wk[:])

            # WqT (one-time): transpose Wq so M = Wq @ kT = (WqT).T @ kT
            wqT_ps = ptp.tile([128, 128], F32, tag="pt")
            nc.tensor.transpose(wqT_ps[:], wq_sb[:], ident[:])
            wqT = wp.tile([128, 128], F32)
            nc.scalar.copy(wqT[:], wqT_ps[:])

            def emit_body():
              for b in range(B_PER_CORE):
                # ---- M[d, a] = Wq @ k^T for this batch ----
                atile = smallp.tile([NA, D], F32, tag="agent")
                nc.sync.dma_start(atile[:], agent[b])

                aT_ps = ptp.tile([128, NA], F32, tag="pt")
                nc.tensor.transpose(aT_ps[:], atile[:], ident[:NA, :NA])
                aT = smallp.tile([128, NA], F32, tag="aT")
                nc.scalar.copy(aT[:], aT_ps[:])

                kT_ps = ptp.tile([128, NA], F32, tag="pt")
                nc.tensor.matmul(kT_ps[:], wk_sb[:], aT[:], start=True, stop=True)
                kT = smallp.tile([128, NA], F32, tag="kT")
                nc.scalar.copy(kT[:], kT_ps[:])

                m_ps = ptp.tile([128, NA], F32, tag="pt")
                nc.tensor.matmul(m_ps[:], wqT[:], kT[:], start=True, stop=True)
                msb = mp.tile([128, NA], F32)
                nc.scalar.copy(msb[:], m_ps[:])

                staging = stagep.tile([128, NSUB * 8], U32, tag="staging")
                stagc = stagep.tile([128, NSUB], U32, tag="stagc")
                # per-sub-block maxes; 8 pad columns so the max_index window
                # [s:s+8] stays in bounds (slots 1-7 are don't-care).
                grouped = stagep.tile([128, NSUB + 8], F32, tag="grouped")
                nc.gpsimd.memset(grouped[:P_USED, :], 0.0)

                ctiles = []
                for j in range(NDMA):
                    ctile = cityp.tile([P_USED, CPP * D], F32)
                    src = city[b, j * CITIES_PER_DMA:(j + 1) * CITIES_PER_DMA, :]
                    dma_eng = nc.gpsimd if USE_SWDGE else nc.sync
                    # two pieces so the first transposes start at half-DMA
                    src2d = src.rearrange("(p n) d -> p (n d)", n=CPP)
                    half = CPP * D // 2
                    dma_eng.dma_start(ctile[:, :half], src2d[:, :half])
                    dma_eng.dma_start(ctile[:, half:], src2d[:, half:])
                    ctiles.append(ctile)

                for g in range(NGRP):
                    # 4 transposes into one PSUM bank, one ACT copy out
                    ctT_ps = ctpp.tile([D, GRP * P_USED], F32, tag="ctT")
                    for t in range(GRP):
                        s = g * GRP + t
                        j, n = divmod(s, CPP)
                        blk = ctiles[j][:, n * D:(n + 1) * D]
                        if USE_F32R:
                            nc.tensor.matmul(
                                ctT_ps[:, t * P_USED:(t + 1) * P_USED].bitcast(F32R),
                                blk.bitcast(F32R),
                                ident[:P_USED, :P_USED].bitcast(F32R),
                                is_transpose=True,
                            )
                        elif TRANS_BF16_ID:
                            nc.tensor.matmul(
                                ctT_ps[:, t * P_USED:(t + 1) * P_USED],
                                blk, ident_bf[:P_USED, :P_USED],
                                is_transpose=True,
                            )
                        else:
                            nc.tensor.transpose(
                                ctT_ps[:, t * P_USED:(t + 1) * P_USED],
                                blk, ident[:P_USED, :P_USED],
                            )
                    ctTs = ctp.tile([D, GRP * P_USED], F32)
                    nc.scalar.copy(ctTs[:], ctT_ps[:])

                    # 4 score matmuls into one PSUM bank
                    sc_ps = psp.tile([P_USED, GRP * NA], F32, tag="sc")
                    if GROUPED_MAXIDX:
                        # rank-1 bias: sc[c, 100t+a] starts at 64*t so the four
                        # sub-blocks' value ranges are disjoint (scores are |x|<~4)
                        nc.tensor.matmul(
                            sc_ps[:], ones_row[:], bias_row[:],
                            start=True, stop=False, skip_group_check=True,
                        )
                    for t in range(GRP):
                        if USE_F32R:
                            nc.tensor.matmul(
                                sc_ps[:, t * NA:(t + 1) * NA],
                                ctTs[:, t * P_USED:(t + 1) * P_USED].bitcast(F32R),
                                msb[:].bitcast(F32R), start=True, stop=True,
                            )
                        else:
                            nc.tensor.matmul(
                                sc_ps[:, t * NA:(t + 1) * NA],
                                ctTs[:, t * P_USED:(t + 1) * P_USED],
                                msb[:],
                                start=not GROUPED_MAXIDX,
                                stop=(not GROUPED_MAXIDX) or t == GRP - 1,
                                skip_group_check=True,
                            )

                    # one 3D reduce_max over the group: [125, 4]
                    nc.vector.reduce_max(
                        grouped[:P_USED, g * GRP:(g + 1) * GRP],
                        sc_ps[:].rearrange("p (t a) -> p t a", a=NA),
                        axis=AX.X,
                    )
                    if GROUPED_MAXIDX:
                        nc.vector.max_index(
                            staging[:P_USED, g * 8:(g + 1) * 8],
                            grouped[:P_USED, g * GRP:g * GRP + 8],
                            sc_ps[:],
                        )
                    else:
                        for t in range(GRP):
                            s = g * GRP + t
                            nc.vector.max_index(
                                staging[:P_USED, s * 8:(s + 1) * 8],
                                grouped[:P_USED, s:s + 8],
                                sc_ps[:, t * NA:(t + 1) * NA],
                            )

                # compact indices [125, NSUB] and store
                if GROUPED_MAXIDX:
                    nc.vector.tensor_copy(
                        stagc[:P_USED, :],
                        staging[:P_USED, :NGRP * 8].rearrange(
                            "p (g e) -> p g e", e=8)[:, :, 0:GRP],
                    )
                else:
                    nc.vector.tensor_copy(
                        stagc[:P_USED, :],
                        staging[:P_USED, :].rearrange("p (s e) -> p s e", e=8)[:, :, 0],
                    )
                nc.sync.dma_start(out[b], stagc[:P_USED, :])

            if reps == 1:
                emit_body()
            else:
                with tc.For_i(0, reps, 1):
                    emit_body()

    nc.finalize()
    return nc


_RUNNER = None


class _Runner:
    """Compile the bass program once; allow repeated execution.

    Mirrors concourse.bass2jax.run_bass_via_pjrt's multi-core branch, but
    keeps the jitted sharded callable so repeat calls don't recompile.
    """

    def __init__(self, reps=1):
        import jax
        from jax.experimental.shard_map import shard_map
        from jax.sharding import Mesh, NamedSharding, PartitionSpec

        import concourse.mybir as _mybir
        from concourse import bass2jax

        self.jax = jax
        self.NamedSharding = NamedSharding
        self.PartitionSpec = PartitionSpec

        bass2jax.install_neuronx_cc_hook()
        nc = build_nc(reps=reps)
        self.nc = nc
        assert nc.dbg_addr is None

        partition_name = (
            nc.partition_id_tensor.name if nc.partition_id_tensor else None
        )
        in_names, out_names, out_avals, zero_outs = [], [], [], []
        for alloc in nc.m.functions[0].allocations:
            if not isinstance(alloc, _mybir.MemoryLocationSet):
                continue
            name = alloc.memorylocations[0].name
            if alloc.kind == "ExternalInput":
                if name != partition_name:
                    in_names.append(name)
            elif alloc.kind == "ExternalOutput":
                shape = tuple(alloc.tensor_shape)
                dtype = _mybir.dt.np(alloc.dtype)
                out_names.append(name)
                out_avals.append(jax.core.ShapedArray(shape, dtype))
                zero_outs.append(np.zeros(shape, dtype))
        n_params = len(in_names)
        n_outs = len(out_avals)
        all_in_names = list(in_names) + list(out_names)
        if partition_name is not None:
            all_in_names.append(partition_name)

        self.in_names = in_names
        self.out_names = out_names
        self.out_avals = out_avals
        self.zero_outs = zero_outs
        self.n_params = n_params

        donate = tuple(range(n_params, n_params + n_outs))

        def _body(*args):
            operands = list(args)
            if partition_name is not None:
                operands.append(bass2jax.partition_id_tensor())
            outs = bass2jax._bass_exec_p.bind(
                *operands,
                out_avals=tuple(out_avals),
                in_names=tuple(all_in_names),
                out_names=tuple(out_names),
                lowering_input_output_aliases=(),
                sim_require_finite=True,
                sim_require_nnan=True,
                nc=nc,
            )
            return tuple(outs)

        devices = jax.devices()[:N_CORES]
        assert len(devices) == N_CORES
        self.mesh = Mesh(np.asarray(devices), ("core",))
        in_specs = (PartitionSpec("core"),) * (n_params + n_outs)
        out_specs = (PartitionSpec("core"),) * n_outs
        self.sharded = jax.jit(
            shard_map(
                _body,
                mesh=self.mesh,
                in_specs=in_specs,
                out_specs=out_specs,
                check_rep=False,
            ),
            donate_argnums=donate,
            keep_unused=True,
        )

    def concat_inputs(self, in_maps):
        return [
            np.concatenate(
                [np.asarray(m[name]) for m in in_maps], axis=0
            )
            for name in self.in_names
        ]

    def device_inputs(self, in_maps):
        """Pre-place concatenated inputs on the mesh (for timing loops)."""
        spec = self.NamedSharding(self.mesh, self.PartitionSpec("core"))
        return [
            self.jax.device_put(a, spec) for a in self.concat_inputs(in_maps)
        ]

    def concat_zeros(self):
        return [
            np.zeros((N_CORES * z.shape[0], *z.shape[1:]), z.dtype)
            for z in self.zero_outs
        ]

    def execute(self, placed_inputs):
        outs = self.sharded(*placed_inputs, *self.concat_zeros())
        self.jax.block_until_ready(outs)
        return outs

    def run(self, in_maps):
        out_arrs = self.execute(self.concat_inputs(in_maps))
        return [
            {
                name: np.asarray(out_arrs[i]).reshape(
                    N_CORES, *self.out_avals[i].shape
                )[c]
                for i, name in enumerate(self.out_names)
            }
            for c in range(N_CORES)
        ]


def _make_runner(reps=1):
    global _RUNNER
    if reps != 1:
        return _Runner(reps=reps)
    if _RUNNER is None:
        _RUNNER = _Runner()
    return _RUNNER


def _unshuffle(raw: np.ndarray) -> np.ndarray:
    """[B_PER_CORE, 125, 40] u32 -> [B_PER_CORE, 5000] city-ordered.

    staging col s = 10*j + n holds city c = 1250*j + 10*p + n.
    """
    if GROUPED_MAXIDX:
        # col s = 4g+t holds 100*t + argmax (disjoint-range bias trick)
        offs = (100 * (np.arange(NSUB) % GRP)).astype(np.uint32)
        raw = raw - offs[None, None, :]
    a = raw.reshape(B_PER_CORE, P_USED, NDMA, CPP)  # [b, p, j, n]
    a = a.transpose(0, 2, 1, 3)                     # [b, j, p, n]
    return a.reshape(B_PER_CORE, NC)                # c = 1250j + 10p + n


def kernel(agent_embed, city_embed, Wq, Wk):
    agent_embed = np.ascontiguousarray(np.asarray(agent_embed, dtype=np.float32))
    city_embed = np.ascontiguousarray(np.asarray(city_embed, dtype=np.float32))
    Wq = np.ascontiguousarray(np.asarray(Wq, dtype=np.float32))
    Wk = np.ascontiguousarray(np.asarray(Wk, dtype=np.float32))

    runner = _make_runner()
    in_maps = [
        {
            "city": city_embed[i * B_PER_CORE:(i + 1) * B_PER_CORE],
            "agent": agent_embed[i * B_PER_CORE:(i + 1) * B_PER_CORE],
            "wq": Wq,
            "wk": Wk,
        }
        for i in range(N_CORES)
    ]
    outs = runner.run(in_maps)
    full = np.empty((B, NC), dtype=np.int32)
    for i in range(N_CORES):
        full[i * B_PER_CORE:(i + 1) * B_PER_CORE] = _unshuffle(
            outs[i]["out"]
        ).astype(np.int32)
    return full


# revision 29
# speedup vs baseline: 1.2463x; 1.2463x over previous
"""Trainium2 Bass kernel for nn_ActionReselector (topk_masking).

reference:
    q = city_embed @ Wq                 [B, NC, D]
    k = agent_embed @ Wk                [B, NA, D]
    scores = q @ k.T / sqrt(D)          [B, NC, NA]
    out = argmax(10*tanh(scores), -1)   [B, NC] int32

Key identities used here:
  - tanh and the positive scales are strictly monotonic -> argmax(scores).
  - scores = city @ (Wq @ k.T) = city @ M with M = [D, NA] tiny.
So per batch we compute M once (three small matmuls), then stream city
through PE (transpose via identity matmul, then scores matmul into grouped
PSUM banks), reduce per-block maxes with 3D reduce_max, and extract indices
with DVE max_index (slot 0 = first index of max, matching jnp.argmax).

Performance structure (per core, 8 batches, ~170us):
  - city loads via SWDGE (gpsimd dma) with 5KB descriptors: engages all 16
    SDMA engines (HWDGE only engaged 5 -> was 2.4x slower).
  - 4 transposes / 4 score matmuls share one PSUM bank each -> one ACT copy
    and one 3D reduce_max per bank instead of four.
  - a rank-1 bf16 bias matmul adds 64*t to sub-block t's scores so the four
    sub-blocks in a bank occupy disjoint value ranges; then ONE max_index
    per bank finds all four argmaxes (host subtracts 100*t).  Exact: scores
    are |x| < ~4 << 64, and slot k's value first-occurs in its own range.
  - everything fp32: float32r is reduced precision, bf16 splits shift the
    argmax for hundreds of rows.  PE (fp32 transposes + matmuls) is the
    bottleneck at ~86% duty.

Sharding: data-parallel over batch B=64 across 8 cores (8 batches/core).
"""

import sys

import numpy as np

try:
    import concourse.bacc as bacc
except ImportError:  # fresh env without the repo on sys.path
    for _p in ("/opt/trn_rl_repo", "/root/.axon_site/_ro/trn_rl_repo"):
        if _p not in sys.path:
            sys.path.insert(0, _p)
    import concourse.bacc as bacc
import concourse.mybir as mybir
import concourse.tile as tile
from concourse import masks

# Problem shapes (hardcoded per contract)
B = 64
NA = 100
NC = 5000
D = 128
N_CORES = 8
B_PER_CORE = B // N_CORES

# City tiling: 4 DMAs per batch, each 1250 cities laid out as
# [125 partitions x 10 cities x 128] so each partition line is 5KB
# contiguous in DRAM.  city index c = 1250*j + 10*p + n.
NDMA = 4
CITIES_PER_DMA = NC // NDMA       # 1250
CPP = 10                          # cities per partition per DMA
P_USED = CITIES_PER_DMA // CPP    # 125
NSUB = NDMA * CPP                 # 40 sub-blocks (125 cities each) per batch
GRP = 4                           # sub-blocks per cityT PSUM bank group
NGRP = NSUB // GRP                # 10 transpose groups per batch
GRP_SC = 5                        # sub-blocks per score PSUM bank
NBANK = NSUB // GRP_SC            # 8 score banks per batch

F32 = mybir.dt.float32
U32 = mybir.dt.uint32
F32R = mybir.dt.float32r
USE_F32R = False  # float32r is reduced precision (TF32-like); corrupts transposes
USE_SWDGE = True
GROUPED_MAXIDX = True
TRANS_BF16_ID = False  # bass asserts mixed fp32/bf16 matmul gives wrong output
BIAS = 64.0
AX = mybir.AxisListType

assert P_USED * GRP * 4 <= 2048        # ctT group fits one PSUM bank
assert NA * GRP_SC * 4 <= 2048         # score bank fits one PSUM bank


def build_nc(reps=1):
    nc = bacc.Bacc(None, target_bir_lowering=False)

    city = nc.dram_tensor("city", [B_PER_CORE, NC, D], F32, kind="ExternalInput")
    agent = nc.dram_tensor("agent", [B_PER_CORE, NA, D], F32, kind="ExternalInput")
    wq = nc.dram_tensor("wq", [D, D], F32, kind="ExternalInput")
    wk = nc.dram_tensor("wk", [D, D], F32, kind="ExternalInput")
    out = nc.dram_tensor("out", [B_PER_CORE, P_USED, NSUB], U32, kind="ExternalOutput")

    with tile.TileContext(nc) as tc:
        with (
            tc.tile_pool(name="const", bufs=1) as constp,
            tc.tile_pool(name="weights", bufs=1) as wp,
            tc.tile_pool(name="cityin", bufs=8) as cityp,
            tc.tile_pool(name="cityT", bufs=4) as ctp,
            tc.tile_pool(name="psumT", bufs=2, space="PSUM") as ptp,
            tc.tile_pool(name="psumCT", bufs=3, space="PSUM") as ctpp,
            tc.tile_pool(name="psumS", bufs=3, space="PSUM") as psp,
            tc.tile_pool(name="mmat", bufs=3) as mp,
            tc.tile_pool(name="small", bufs=3) as smallp,
            tc.tile_pool(name="stage", bufs=3) as stagep,
        ):
            ident = constp.tile([128, 128], F32)
            masks.make_identity(nc, ident[:])
            ident_bf = constp.tile([128, 128], mybir.dt.bfloat16)
            masks.make_identity(nc, ident_bf[:])

            BF16 = mybir.dt.bfloat16
            ones_row = constp.tile([1, P_USED], BF16)
            nc.gpsimd.memset(ones_row[:], 1.0)
            bias_row = constp.tile([1, GRP_SC * NA], BF16)
            for _t in range(GRP_SC):
                nc.gpsimd.memset(bias_row[:, _t * NA:(_t + 1) * NA], BIAS * _t)

            wq_sb = wp.tile([128, 128], F32)
            nc.sync.dma_start(wq_sb[:], wq[:])
            wk_sb = wp.tile([128, 128], F32)
            nc.sync.dma_start(wk_sb[:], wk[:])

            # WqT (one-time): transpose Wq so M = Wq @ kT = (WqT).T @ kT
            wqT_ps = ptp.tile([128, 128], F32, tag="pt")
            nc.tensor.transpose(wqT_ps[:], wq_sb[:], ident[:])
            wqT = wp.tile([128, 128], F32)
            nc.scalar.copy(wqT[:], wqT_ps[:])

            def emit_body():
              for b in range(B_PER_CORE):
                # ---- M[d, a] = Wq @ k^T for this batch ----
                atile = smallp.tile([NA, D], F32, tag="agent")
                nc.sync.dma_start(atile[:], agent[b])

                aT_ps = ptp.tile([128, NA], F32, tag="pt")
                nc.tensor.transpose(aT_ps[:], atile[:], ident[:NA, :NA])
                aT = smallp.tile([128, NA], F32, tag="aT")
                nc.scalar.copy(aT[:], aT_ps[:])

                kT_ps = ptp.tile([128, NA], F32, tag="pt")
                nc.tensor.matmul(kT_ps[:], wk_sb[:], aT[:], start=True, stop=True)
                kT = smallp.tile([128, NA], F32, tag="kT")
                nc.scalar.copy(kT[:], kT_ps[:])

                m_ps = ptp.tile([128, NA], F32, tag="pt")
                nc.tensor.matmul(m_ps[:], wqT[:], kT[:], start=True, stop=True)
                msb = mp.tile([128, NA], F32)
                nc.scalar.copy(msb[:], m_ps[:])

                staging = stagep.tile([128, NSUB * 8], U32, tag="staging")
                stagc = stagep.tile([128, NSUB], U32, tag="stagc")
                # per-sub-block maxes; 8 pad columns so the max_index window
                # [s:s+8] stays in bounds (slots 1-7 are don't-care).
                grouped = stagep.tile([128, NSUB + 8], F32, tag="grouped")
                nc.gpsimd.memset(grouped[:P_USED, :], 0.0)

                ctiles = []
                for j in range(NDMA):
                    ctile = cityp.tile([P_USED, CPP * D], F32)
                    src = city[b, j * CITIES_PER_DMA:(j + 1) * CITIES_PER_DMA, :]
                    dma_eng = nc.gpsimd if USE_SWDGE else nc.sync
                    # two pieces so the first transposes start at half-DMA
                    src2d = src.rearrange("(p n) d -> p (n d)", n=CPP)
                    half = CPP * D // 2
                    dma_eng.dma_start(ctile[:, :half], src2d[:, :half])
                    dma_eng.dma_start(ctile[:, half:], src2d[:, half:])
                    ctiles.append(ctile)

                ctTs_by_g = {}
                sc_ps = None
                for s in range(NSUB):
                    g, t = divmod(s, GRP)
                    h, u = divmod(s, GRP_SC)
                    if t == 0:
                        # 4 transposes into one PSUM bank, one ACT copy out
                        ctT_ps = ctpp.tile([D, GRP * P_USED], F32, tag="ctT")
                        for tt in range(GRP):
                            ss = g * GRP + tt
                            j, n = divmod(ss, CPP)
                            blk = ctiles[j][:, n * D:(n + 1) * D]
                            nc.tensor.transpose(
                                ctT_ps[:, tt * P_USED:(tt + 1) * P_USED],
                                blk, ident[:P_USED, :P_USED],
                            )
                        ctTs = ctp.tile([D, GRP * P_USED], F32)
                        nc.scalar.copy(ctTs[:], ctT_ps[:])
                        ctTs_by_g[g] = ctTs

                    if u == 0:
                        # new score bank: rank-1 bias gives sub-block u the
                        # range 64*u +- ~4 so the five ranges are disjoint
                        sc_ps = psp.tile([P_USED, GRP_SC * NA], F32, tag="sc")
                        nc.tensor.matmul(
                            sc_ps[:], ones_row[:], bias_row[:],
                            start=True, stop=False, skip_group_check=True,
                        )
                    nc.tensor.matmul(
                        sc_ps[:, u * NA:(u + 1) * NA],
                        ctTs_by_g[g][:, t * P_USED:(t + 1) * P_USED],
                        msb[:],
                        start=False, stop=u == GRP_SC - 1,
                        skip_group_check=True,
                    )
                    if u == GRP_SC - 1:
                        # one 3D reduce_max over the bank: [125, 5]
                        nc.vector.reduce_max(
                            grouped[:P_USED, h * GRP_SC:(h + 1) * GRP_SC],
                            sc_ps[:].rearrange("p (t a) -> p t a", a=NA),
                            axis=AX.X,
                        )
                        nc.vector.max_index(
                            staging[:P_USED, h * 8:(h + 1) * 8],
                            grouped[:P_USED, h * GRP_SC:h * GRP_SC + 8],
                            sc_ps[:],
                        )

                # compact indices [125, NSUB] and store
                nc.vector.tensor_copy(
                    stagc[:P_USED, :],
                    staging[:P_USED, :NBANK * 8].rearrange(
                        "p (h e) -> p h e", e=8)[:, :, 0:GRP_SC],
                )
                nc.sync.dma_start(out[b], stagc[:P_USED, :])

            if reps == 1:
                emit_body()
            else:
                with tc.For_i(0, reps, 1):
                    emit_body()

    nc.finalize()
    return nc


_RUNNER = None


class _Runner:
    """Compile the bass program once; allow repeated execution.

    Mirrors concourse.bass2jax.run_bass_via_pjrt's multi-core branch, but
    keeps the jitted sharded callable so repeat calls don't recompile.
    """

    def __init__(self, reps=1):
        import jax
        from jax.experimental.shard_map import shard_map
        from jax.sharding import Mesh, NamedSharding, PartitionSpec

        import concourse.mybir as _mybir
        from concourse import bass2jax

        self.jax = jax
        self.NamedSharding = NamedSharding
        self.PartitionSpec = PartitionSpec

        bass2jax.install_neuronx_cc_hook()
        nc = build_nc(reps=reps)
        self.nc = nc
        assert nc.dbg_addr is None

        partition_name = (
            nc.partition_id_tensor.name if nc.partition_id_tensor else None
        )
        in_names, out_names, out_avals, zero_outs = [], [], [], []
        for alloc in nc.m.functions[0].allocations:
            if not isinstance(alloc, _mybir.MemoryLocationSet):
                continue
            name = alloc.memorylocations[0].name
            if alloc.kind == "ExternalInput":
                if name != partition_name:
                    in_names.append(name)
            elif alloc.kind == "ExternalOutput":
                shape = tuple(alloc.tensor_shape)
                dtype = _mybir.dt.np(alloc.dtype)
                out_names.append(name)
                out_avals.append(jax.core.ShapedArray(shape, dtype))
                zero_outs.append(np.zeros(shape, dtype))
        n_params = len(in_names)
        n_outs = len(out_avals)
        all_in_names = list(in_names) + list(out_names)
        if partition_name is not None:
            all_in_names.append(partition_name)

        self.in_names = in_names
        self.out_names = out_names
        self.out_avals = out_avals
        self.zero_outs = zero_outs
        self.n_params = n_params

        donate = tuple(range(n_params, n_params + n_outs))

        def _body(*args):
            operands = list(args)
            if partition_name is not None:
                operands.append(bass2jax.partition_id_tensor())
            outs = bass2jax._bass_exec_p.bind(
                *operands,
                out_avals=tuple(out_avals),
                in_names=tuple(all_in_names),
                out_names=tuple(out_names),
                lowering_input_output_aliases=(),
                sim_require_finite=True,
                sim_require_nnan=True,
                nc=nc,
            )
            return tuple(outs)

        devices = jax.devices()[:N_CORES]
        assert len(devices) == N_CORES
        self.mesh = Mesh(np.asarray(devices), ("core",))
        in_specs = (PartitionSpec("core"),) * (n_params + n_outs)
        out_specs = (PartitionSpec("core"),) * n_outs
        self.sharded = jax.jit(
            shard_map(
                _body,
                mesh=self.mesh,
                in_specs=in_specs,
                out_specs=out_specs,
                check_rep=False,
            ),
            donate_argnums=donate,
            keep_unused=True,
        )

    def concat_inputs(self, in_maps):
        return [
            np.concatenate(
                [np.asarray(m[name]) for m in in_maps], axis=0
            )
            for name in self.in_names
        ]

    def device_inputs(self, in_maps):
        """Pre-place concatenated inputs on the mesh (for timing loops)."""
        spec = self.NamedSharding(self.mesh, self.PartitionSpec("core"))
        return [
            self.jax.device_put(a, spec) for a in self.concat_inputs(in_maps)
        ]

    def concat_zeros(self):
        return [
            np.zeros((N_CORES * z.shape[0], *z.shape[1:]), z.dtype)
            for z in self.zero_outs
        ]

    def execute(self, placed_inputs):
        outs = self.sharded(*placed_inputs, *self.concat_zeros())
        self.jax.block_until_ready(outs)
        return outs

    def run(self, in_maps):
        out_arrs = self.execute(self.concat_inputs(in_maps))
        return [
            {
                name: np.asarray(out_arrs[i]).reshape(
                    N_CORES, *self.out_avals[i].shape
                )[c]
                for i, name in enumerate(self.out_names)
            }
            for c in range(N_CORES)
        ]


def _make_runner(reps=1):
    global _RUNNER
    if reps != 1:
        return _Runner(reps=reps)
    if _RUNNER is None:
        _RUNNER = _Runner()
    return _RUNNER


def _unshuffle(raw: np.ndarray) -> np.ndarray:
    """[B_PER_CORE, 125, 40] u32 -> [B_PER_CORE, 5000] city-ordered.

    staging col s = 10*j + n holds city c = 1250*j + 10*p + n.
    """
    if GROUPED_MAXIDX:
        # col s = 5h+u holds 100*u + argmax (disjoint-range bias trick)
        offs = (100 * (np.arange(NSUB) % GRP_SC)).astype(np.uint32)
        raw = raw - offs[None, None, :]
    a = raw.reshape(B_PER_CORE, P_USED, NDMA, CPP)  # [b, p, j, n]
    a = a.transpose(0, 2, 1, 3)                     # [b, j, p, n]
    return a.reshape(B_PER_CORE, NC)                # c = 1250j + 10p + n


def kernel(agent_embed, city_embed, Wq, Wk):
    agent_embed = np.ascontiguousarray(np.asarray(agent_embed, dtype=np.float32))
    city_embed = np.ascontiguousarray(np.asarray(city_embed, dtype=np.float32))
    Wq = np.ascontiguousarray(np.asarray(Wq, dtype=np.float32))
    Wk = np.ascontiguousarray(np.asarray(Wk, dtype=np.float32))

    runner = _make_runner()
    in_maps = [
        {
            "city": city_embed[i * B_PER_CORE:(i + 1) * B_PER_CORE],
            "agent": agent_embed[i * B_PER_CORE:(i + 1) * B_PER_CORE],
            "wq": Wq,
            "wk": Wk,
        }
        for i in range(N_CORES)
    ]
    outs = runner.run(in_maps)
    full = np.empty((B, NC), dtype=np.int32)
    for i in range(N_CORES):
        full[i * B_PER_CORE:(i + 1) * B_PER_CORE] = _unshuffle(
            outs[i]["out"]
        ).astype(np.int32)
    return full
